# revision 6
# baseline (speedup 1.0000x reference)
"""MoE layer (B=4,S=2048,D=1024,I=4096,E=8,top_k=2) on 8 TRN2 NeuronCores.

Strategy: expert-parallel sparse dispatch.
 - Host: router matmul (tiny), top-k + softmax gates, gather tokens per expert.
 - Device (core e == expert e): yT = (gelu(x @ W1) @ W2 + b2) * gate, with
   x/W in bf16 on the TensorEngine, fp32 PSUM accumulation, token dim padded
   to a multiple of 128 and processed in 512-wide chunks.
 - Host: scatter-add the K=2 gated expert outputs back to [B,S,D].

v2: consolidated head DMAs (one 3D-AP descriptor for x chunk-0, w1 in 5
column-region descriptors) — the DMA-trigger instructions on the Sync queue
cost ~0.6us EACH to issue, so the old 41-descriptor head didn't have its
data landed until ~16.6us while warmup ended at ~12.8us; the resulting
3.8us PE idle crossed the HAM MID window and the first ~45 real matmuls ran
at 1.2GHz. Warmup count tuned to end at data-ready.
"""

import os

import ml_dtypes
import numpy as np

import concourse.bass as bass
import concourse.bacc as bacc
import concourse.mybir as mybir
import concourse.tile as tile
from concourse.bass_utils import run_bass_kernel_spmd

BF16 = mybir.dt.bfloat16
F32 = mybir.dt.float32
P = 128
N_CORES = 8
NDUMMY = 11  # HAM warmup matmuls (~427ns each cold, spans until head DMAs land)

# Filled with the profiled exec time (ns) of the last run when
# BASS_KERNEL_TRACE=1 is set in the environment (used by test.py).
LAST_EXEC_NS = None
LAST_RESULTS = None

_cache: dict = {}


def _chunks_for(C: int) -> list[int]:
    """[512, ..., remainder]. Measured faster than equal-width chunks:
    N=512 matmuls hit the 216ns streaming bound and the N=128 tail runs at
    ~56ns/MM (FWL hides LDWEIGHTS), while e.g. N=448 matmuls miss the
    N/2.4+2.5ns model."""
    chunks = [512] * (C // 512)
    if C % 512:
        chunks.append(C % 512)
    return chunks


# w1 column regions: first small so the first m1 groups' weights land early,
# rest sized 512B+ per DMA run. Region r covers output tiles i with
# 128*i in [a, b).
W1_REGIONS = [(0, 384), (384, 1024), (1024, 2048), (2048, 3072), (3072, 4096)]


def _build(C: int, D: int, I: int):
    """Per-core FFN program: one expert, C token slots (multiple of 128)."""
    KD = D // P  # k-tiles for contraction over D
    KI = I // P  # k-tiles for contraction over I
    ND = D // P  # output row tiles

    nc = bacc.Bacc()
    xT = nc.declare_dram_parameter("xT", [D, C], BF16, isOutput=False)
    w1 = nc.declare_dram_parameter("w1", [D, I], BF16, isOutput=False)
    b1 = nc.declare_dram_parameter("b1", [P, I // P], F32, isOutput=False)
    w2 = nc.declare_dram_parameter("w2", [I, D], BF16, isOutput=False)
    b2 = nc.declare_dram_parameter("b2", [P, D // P], F32, isOutput=False)
    g = nc.declare_dram_parameter("g", [P, C], F32, isOutput=False)
    yT = nc.declare_dram_parameter("yT", [D, C], F32, isOutput=True)

    xTr3 = xT[:].rearrange("(k p) c -> p k c", p=P)  # [128, KD, C]
    w1r3 = w1[:].rearrange("(k p) i -> p k i", p=P)  # [128, KD, I]
    w2r3 = w2[:].rearrange("(k p) d -> p k d", p=P)  # [128, KI, D]
    yTr = yT[:].rearrange("(k p) c -> k p c", p=P)

    def w1_slice(w1_sb, i, k):
        """Stationary [128,128] for m1 output tile i, contraction tile k."""
        col = i * P
        for r, (a, b) in enumerate(W1_REGIONS):
            if a <= col < b:
                return w1_sb[r][:, k, col - a : col - a + P]
        raise AssertionError(col)

    with tile.TileContext(nc) as tc:
        with (
            tc.tile_pool(name="wpool", bufs=1) as wpool,
            tc.tile_pool(name="cpool", bufs=1) as cpool,
            tc.tile_pool(name="xpool", bufs=2) as xpool,
            tc.tile_pool(name="hpool", bufs=1) as hpool,
            tc.tile_pool(name="ypool", bufs=4) as ypool,
            tc.tile_pool(name="pspool", bufs=8, space="PSUM") as pspool,
        ):
            chunks = _chunks_for(C)
            # Head DMAs. DMAs on one queue serialize (each PSEUDO_DMA blocks
            # the queue for its transfer at ~400GB/s) and CROSS-queue DMAs
            # contend for the same engines (measured: x0 on the ACT queue
            # starved to ~90GB/s behind the w1 stream on Sync). So: keep
            # everything on Sync, 2D per-k descriptors, with x chunk-0 and
            # the first (shrunk, 384-col) w1 region interleaved up front —
            # 1.77MB of head data lands ~12us in, vs 3MB/16.5us before.
            cw0 = chunks[0]
            xTr2 = xT[:].rearrange("(k p) c -> k p c", p=P)
            w1r2 = w1[:].rearrange("(k p) i -> k p i", p=P)
            x0_t = xpool.tile([P, KD, cw0], BF16, tag="x")
            w1_sb = []
            a0, b0 = W1_REGIONS[0]
            w1a_t = wpool.tile([P, KD, b0 - a0], BF16, tag="w1_0")
            w1_sb.append(w1a_t)
            for k in range(KD):
                nc.sync.dma_start(out=x0_t[:, k, :], in_=xTr2[k][:, :cw0])
                nc.sync.dma_start(out=w1a_t[:, k, :], in_=w1r2[k][:, a0:b0])
            b1_sb = cpool.tile([P, I // P], F32, tag="b1")
            nc.sync.dma_start(out=b1_sb[:], in_=b1[:])
            for r, (a, b) in enumerate(W1_REGIONS[1:], start=1):
                t = wpool.tile([P, KD, b - a], BF16, tag=f"w1_{r}")
                nc.sync.dma_start(out=t[:], in_=w1r3[:, :, a:b])
                w1_sb.append(t)
            # The Activation encoding fits a single sync wait. Every gelu's
            # PSUM RAW wait (PE sem) dominates its h-slot WAR tick, so the
            # only extra wait a gelu could need is the b1 DMA — absorb it
            # once with a 1-element warm-up copy so ACT's vector clock has
            # observed that DMA before the first real gelu.
            warm = cpool.tile([1, 1], F32, tag="warm")
            warm2 = cpool.tile([1, 1], F32, tag="warm2")
            nc.scalar.copy(warm[:], b1_sb[:1, :1])

            # HAM warm-up: dummy matmuls on zeroed scratch while the head
            # DMAs stream, so real matmuls start at 2.4 GHz instead of
            # paying the 1.2 GHz cold window. Count tuned so the dummies
            # end right as the head data lands (each ~427ns at 1.2GHz).
            scratch = cpool.tile([P, 512], BF16, tag="scratch")
            nc.gpsimd.memset(scratch[:], 0.0)
            left = NDUMMY
            while left > 0:
                grp = min(8, left)
                pw = pspool.tile([P, 512], F32, tag="ps")
                for k in range(grp):
                    nc.tensor.matmul(
                        pw[:],
                        scratch[:, :P],
                        scratch[:],
                        start=(k == 0),
                        stop=(k == grp - 1),
                    )
                left -= grp

            # W2/b2/g are not needed until the first m2 phase — their DMAs
            # are emitted after chunk-0 m1 below so they don't queue ahead of
            # the chunk-0 x tiles in the DMA FIFOs (measured 55 us PE stall).
            w2_sb = []
            b2_dve = None
            g_dve = None

            def _load_phase2():
                for q in range(KI // KD):
                    t = wpool.tile([P, KD, D], BF16, tag=f"w2_{q}")
                    nc.sync.dma_start(
                        out=t[:], in_=w2r3[:, q * KD : (q + 1) * KD, :]
                    )
                    w2_sb.append(t)
                b2_sb = cpool.tile([P, D // P], F32, tag="b2")
                nc.sync.dma_start(out=b2_sb[:], in_=b2[:])
                g_sb = cpool.tile([P, C], F32, tag="g")
                nc.sync.dma_start(out=g_sb[:], in_=g[:])
                b2_stage = cpool.tile([P, D // P], F32, tag="b2v")
                nc.vector.tensor_copy(b2_stage[:], b2_sb[:])
                g_stage = cpool.tile([P, C], F32, tag="gv")
                nc.vector.tensor_copy(g_stage[:], g_sb[:])
                return b2_stage, g_stage

            c0 = 0
            prev_h_last = None
            for ci, cw in enumerate(chunks):
                if ci == 0:
                    x_t = x0_t
                else:
                    x_t = xpool.tile([P, KD, cw], BF16, tag="x")
                    nc.sync.dma_start(out=x_t[:], in_=xTr3[:, :, c0 : c0 + cw])
                # hT = gelu(x @ W1 + b1), tiled [128 of I, cw]
                if prev_h_last is not None:
                    # Advance ACT's observed self-tick past ALL of the
                    # previous chunk's gelus so the h-tile WAW deps below
                    # don't each need their own (second) sync wait.
                    nc.scalar.copy(warm[:], prev_h_last[:1, :1])
                    nc.scalar.copy(warm2[:], warm[:])
                h_sb = []
                for i in range(KI):
                    ps = pspool.tile([P, cw], F32, tag="ps")
                    for k in range(KD):
                        nc.tensor.matmul(
                            ps[:],
                            w1_slice(w1_sb, i, k),
                            x_t[:, k, :],
                            start=(k == 0),
                            stop=(k == KD - 1),
                        )
                    ht = hpool.tile([P, cw], BF16, tag=f"h_{i}")
                    nc.scalar.activation(
                        ht[:],
                        ps[:],
                        mybir.ActivationFunctionType.Gelu,
                        bias=b1_sb[:, i : i + 1],
                    )
                    h_sb.append(ht)
                prev_h_last = h_sb[-1]
                if g_dve is None:
                    b2_dve, g_dve = _load_phase2()
                # yT = (hT' @ W2 + b2) * g, tiled [128 of D, cw]
                for d in range(ND):
                    ps = pspool.tile([P, cw], F32, tag="ps")
                    for k in range(KI):
                        nc.tensor.matmul(
                            ps[:],
                            w2_sb[k // KD][:, k % KD, d * P : (d + 1) * P],
                            h_sb[k][:],
                            start=(k == 0),
                            stop=(k == KI - 1),
                        )
                    yt = ypool.tile([P, cw], F32, tag="y")
                    nc.vector.scalar_tensor_tensor(
                        out=yt[:],
                        in0=ps[:],
                        scalar=b2_dve[:, d : d + 1],
                        in1=g_dve[:, c0 : c0 + cw],
                        op0=mybir.AluOpType.add,
                        op1=mybir.AluOpType.mult,
                    )
                    nc.sync.dma_start(out=yTr[d][:, c0 : c0 + cw], in_=yt[:])
                c0 += cw
    nc.compile()
    return nc


def kernel(**inputs) -> np.ndarray:
    global LAST_EXEC_NS, LAST_RESULTS
    x = np.asarray(inputs["x"], dtype=np.float32)
    Wr = np.asarray(inputs["Wr"], dtype=np.float32)
    br = np.asarray(inputs["br"], dtype=np.float32)
    W1 = np.asarray(inputs["W1"], dtype=np.float32)
    b1 = np.asarray(inputs["b1"], dtype=np.float32)
    W2 = np.asarray(inputs["W2"], dtype=np.float32)
    b2 = np.asarray(inputs["b2"], dtype=np.float32)
    K = int(np.asarray(inputs["top_k"]))

    B, S, D = x.shape
    E = Wr.shape[0]
    I = W1.shape[2]
    T = B * S
    xf = x.reshape(T, D)

    # Router (tiny) on host: logits -> top-k (desc, ties -> lower index,
    # matching jax.lax.top_k) -> softmax over the selected k.
    logits = xf @ Wr.T + br
    order = np.argsort(-logits, axis=-1, kind="stable")[:, :K]
    topv = np.take_along_axis(logits, order, axis=-1)
    exv = np.exp(topv - topv.max(axis=-1, keepdims=True))
    gates = (exv / exv.sum(axis=-1, keepdims=True)).astype(np.float32)

    toks, gvals = [], []
    for e in range(E):
        sel = order == e
        tok = np.nonzero(sel.any(axis=-1))[0]
        kidx = np.argmax(sel[tok], axis=-1)
        toks.append(tok)
        gvals.append(gates[tok, kidx].astype(np.float32))

    maxc = max(max(len(t) for t in toks), P)
    C = ((maxc + P - 1) // P) * P

    key = (C, D, I)
    if key not in _cache:
        _cache[key] = _build(C, D, I)
    nc = _cache[key]

    bf = ml_dtypes.bfloat16
    in_maps = []
    for e in range(E):
        n = len(toks[e])
        xTe = np.zeros((D, C), dtype=bf)
        if n:
            xTe[:, :n] = xf[toks[e]].T.astype(bf)
        ge = np.zeros((P, C), dtype=np.float32)
        if n:
            ge[:, :n] = gvals[e][None, :]
        in_maps.append(
            {
                "xT": xTe,
                "w1": np.ascontiguousarray(W1[e].astype(bf)),
                "b1": np.ascontiguousarray(b1[e].reshape(I // P, P).T),
                "w2": np.ascontiguousarray(W2[e].astype(bf)),
                "b2": np.ascontiguousarray(b2[e].reshape(D // P, P).T),
                "g": ge,
            }
        )

    trace = bool(int(os.environ.get("BASS_KERNEL_TRACE", "0")))
    if trace:
        try:
            from antenv.axon_hooks import get_axon_ntff_profile_hook  # noqa: F401
        except ImportError:
            trace = False
    res = run_bass_kernel_spmd(
        nc, in_maps, core_ids=list(range(N_CORES)), trace=trace
    )
    LAST_EXEC_NS = res.exec_time_ns
    LAST_RESULTS = res

    out = np.zeros((T, D), dtype=np.float32)
    for e in range(E):
        n = len(toks[e])
        if n:
            out[toks[e]] += res.results[e]["yT"][:, :n].T
    return out.reshape(B, S, D)


# revision 10
# speedup vs baseline: 1.0060x; 1.0060x over previous
"""MoE layer (B=4,S=2048,D=1024,I=4096,E=8,top_k=2) on 8 TRN2 NeuronCores.

Strategy: expert-parallel sparse dispatch.
 - Host: router matmul (tiny), top-k + softmax gates, gather tokens per expert.
 - Device (core e == expert e): yT = (gelu(x @ W1) @ W2 + b2) * gate, with
   x/W in bf16 on the TensorEngine, fp32 PSUM accumulation, token dim padded
   to a multiple of 128 and processed in 512-wide chunks.
 - Host: scatter-add the K=2 gated expert outputs back to [B,S,D].

v2: consolidated head DMAs (one 3D-AP descriptor for x chunk-0, w1 in 5
column-region descriptors) — the DMA-trigger instructions on the Sync queue
cost ~0.6us EACH to issue, so the old 41-descriptor head didn't have its
data landed until ~16.6us while warmup ended at ~12.8us; the resulting
3.8us PE idle crossed the HAM MID window and the first ~45 real matmuls ran
at 1.2GHz. Warmup count tuned to end at data-ready.
"""

import os

import ml_dtypes
import numpy as np

import concourse.bass as bass
import concourse.bacc as bacc
import concourse.mybir as mybir
import concourse.tile as tile
from concourse.bass_utils import run_bass_kernel_spmd

BF16 = mybir.dt.bfloat16
F32 = mybir.dt.float32
P = 128
N_CORES = 8
NDUMMY = 11  # HAM warmup matmuls (~427ns each cold, spans until head DMAs land)

# Filled with the profiled exec time (ns) of the last run when
# BASS_KERNEL_TRACE=1 is set in the environment (used by test.py).
LAST_EXEC_NS = None
LAST_RESULTS = None

_cache: dict = {}


def _chunks_for(C: int) -> list[int]:
    """[512, ..., remainder]. Measured faster than equal-width chunks:
    N=512 matmuls hit the 216ns streaming bound and the N=128 tail runs at
    ~56ns/MM (FWL hides LDWEIGHTS), while e.g. N=448 matmuls miss the
    N/2.4+2.5ns model."""
    chunks = [512] * (C // 512)
    if C % 512:
        chunks.append(C % 512)
    return chunks


# w1 column regions: first small so the first m1 groups' weights land early,
# rest sized 512B+ per DMA run. Region r covers output tiles i with
# 128*i in [a, b).
W1_REGIONS = [(0, 384), (384, 1024), (1024, 2048), (2048, 3072), (3072, 4096)]


def _build(C: int, D: int, I: int):
    """Per-core FFN program: one expert, C token slots (multiple of 128)."""
    KD = D // P  # k-tiles for contraction over D
    KI = I // P  # k-tiles for contraction over I
    ND = D // P  # output row tiles

    nc = bacc.Bacc()
    xT = nc.declare_dram_parameter("xT", [D, C], BF16, isOutput=False)
    w1 = nc.declare_dram_parameter("w1", [D, I], BF16, isOutput=False)
    b1 = nc.declare_dram_parameter("b1", [P, I // P], F32, isOutput=False)
    w2 = nc.declare_dram_parameter("w2", [I, D], BF16, isOutput=False)
    b2 = nc.declare_dram_parameter("b2", [P, D // P], F32, isOutput=False)
    g = nc.declare_dram_parameter("g", [P, C], F32, isOutput=False)
    yT = nc.declare_dram_parameter("yT", [D, C], F32, isOutput=True)

    yTr = yT[:].rearrange("(k p) c -> k p c", p=P)

    def w1_slice(w1_sb, i, k):
        """Stationary [128,128] for m1 output tile i, contraction tile k."""
        col = i * P
        for r, (a, b) in enumerate(W1_REGIONS):
            if a <= col < b:
                return w1_sb[r][:, k, col - a : col - a + P]
        raise AssertionError(col)

    with tile.TileContext(nc) as tc:
        with (
            tc.tile_pool(name="wpool", bufs=1) as wpool,
            tc.tile_pool(name="cpool", bufs=1) as cpool,
            tc.tile_pool(name="xpool", bufs=2) as xpool,
            tc.tile_pool(name="hpool", bufs=1) as hpool,
            tc.tile_pool(name="ypool", bufs=4) as ypool,
            tc.tile_pool(name="pspool", bufs=8, space="PSUM") as pspool,
        ):
            chunks = _chunks_for(C)
            # Head DMAs. All on the Sync queue as per-k 2D descriptors:
            # 2D per-k transfers walk DRAM rows sequentially and run at
            # ~350GB/s, while a consolidated 3D [p,k,cols] descriptor jumps
            # 1MB between rows and measured ~3x slower. x chunk-0 and the
            # first (shrunk, 384-col) w1 region interleave up front so the
            # 1.77MB gating the first real matmul lands ~12.5us in (vs
            # 3MB/16.5us for the old 1024-col first region), with the HAM
            # warmup spanning the wait.
            cw0 = chunks[0]
            xTr2 = xT[:].rearrange("(k p) c -> k p c", p=P)
            w1r2 = w1[:].rearrange("(k p) i -> k p i", p=P)
            x0_t = xpool.tile([P, KD, cw0], BF16, tag="x")
            w1_sb = []
            a0, b0 = W1_REGIONS[0]
            w1a_t = wpool.tile([P, KD, b0 - a0], BF16, tag="w1_0")
            w1_sb.append(w1a_t)
            for k in range(KD):
                nc.sync.dma_start(out=x0_t[:, k, :], in_=xTr2[k][:, :cw0])
                nc.sync.dma_start(out=w1a_t[:, k, :], in_=w1r2[k][:, a0:b0])
            b1_sb = cpool.tile([P, I // P], F32, tag="b1")
            nc.sync.dma_start(out=b1_sb[:], in_=b1[:])
            for r, (a, b) in enumerate(W1_REGIONS[1:], start=1):
                t = wpool.tile([P, KD, b - a], BF16, tag=f"w1_{r}")
                for k in range(KD):
                    nc.sync.dma_start(out=t[:, k, :], in_=w1r2[k][:, a:b])
                w1_sb.append(t)
            # The Activation encoding fits a single sync wait. Every gelu's
            # PSUM RAW wait (PE sem) dominates its h-slot WAR tick, so the
            # only extra wait a gelu could need is the b1 DMA — absorb it
            # once with a 1-element warm-up copy so ACT's vector clock has
            # observed that DMA before the first real gelu.
            warm = cpool.tile([1, 1], F32, tag="warm")
            warm2 = cpool.tile([1, 1], F32, tag="warm2")
            nc.scalar.copy(warm[:], b1_sb[:1, :1])

            # HAM warm-up: dummy matmuls on zeroed scratch while the head
            # DMAs stream, so real matmuls start at 2.4 GHz instead of
            # paying the 1.2 GHz cold window. Count tuned so the dummies
            # end right as the head data lands (each ~427ns at 1.2GHz).
            scratch = cpool.tile([P, 512], BF16, tag="scratch")
            nc.gpsimd.memset(scratch[:], 0.0)
            left = NDUMMY
            while left > 0:
                grp = min(8, left)
                pw = pspool.tile([P, 512], F32, tag="ps")
                for k in range(grp):
                    nc.tensor.matmul(
                        pw[:],
                        scratch[:, :P],
                        scratch[:],
                        start=(k == 0),
                        stop=(k == grp - 1),
                    )
                left -= grp

            # W2/b2/g are not needed until the first m2 phase — their DMAs
            # are emitted after chunk-0 m1 below so they don't queue ahead of
            # the chunk-0 x tiles in the DMA FIFOs (measured 55 us PE stall).
            w2_sb = []
            b2_dve = None
            g_dve = None

            w2r2 = w2[:].rearrange("(k p) d -> k p d", p=P)

            def _load_phase2():
                for q in range(KI // KD):
                    t = wpool.tile([P, KD, D], BF16, tag=f"w2_{q}")
                    for k in range(KD):
                        nc.sync.dma_start(
                            out=t[:, k, :], in_=w2r2[q * KD + k]
                        )
                    w2_sb.append(t)
                b2_sb = cpool.tile([P, D // P], F32, tag="b2")
                nc.sync.dma_start(out=b2_sb[:], in_=b2[:])
                g_sb = cpool.tile([P, C], F32, tag="g")
                nc.sync.dma_start(out=g_sb[:], in_=g[:])
                b2_stage = cpool.tile([P, D // P], F32, tag="b2v")
                nc.vector.tensor_copy(b2_stage[:], b2_sb[:])
                g_stage = cpool.tile([P, C], F32, tag="gv")
                nc.vector.tensor_copy(g_stage[:], g_sb[:])
                return b2_stage, g_stage

            c0 = 0
            prev_h_last = None
            for ci, cw in enumerate(chunks):
                if ci == 0:
                    x_t = x0_t
                else:
                    x_t = xpool.tile([P, KD, cw], BF16, tag="x")
                    for k in range(KD):
                        nc.sync.dma_start(
                            out=x_t[:, k, :], in_=xTr2[k][:, c0 : c0 + cw]
                        )
                # hT = gelu(x @ W1 + b1), tiled [128 of I, cw]
                if prev_h_last is not None:
                    # Advance ACT's observed self-tick past ALL of the
                    # previous chunk's gelus so the h-tile WAW deps below
                    # don't each need their own (second) sync wait.
                    nc.scalar.copy(warm[:], prev_h_last[:1, :1])
                    nc.scalar.copy(warm2[:], warm[:])
                h_sb = []
                for i in range(KI):
                    ps = pspool.tile([P, cw], F32, tag="ps")
                    for k in range(KD):
                        nc.tensor.matmul(
                            ps[:],
                            w1_slice(w1_sb, i, k),
                            x_t[:, k, :],
                            start=(k == 0),
                            stop=(k == KD - 1),
                        )
                    ht = hpool.tile([P, cw], BF16, tag=f"h_{i}")
                    nc.scalar.activation(
                        ht[:],
                        ps[:],
                        mybir.ActivationFunctionType.Gelu,
                        bias=b1_sb[:, i : i + 1],
                    )
                    h_sb.append(ht)
                prev_h_last = h_sb[-1]
                if g_dve is None:
                    b2_dve, g_dve = _load_phase2()
                # yT = (hT' @ W2 + b2) * g, tiled [128 of D, cw]
                for d in range(ND):
                    ps = pspool.tile([P, cw], F32, tag="ps")
                    for k in range(KI):
                        nc.tensor.matmul(
                            ps[:],
                            w2_sb[k // KD][:, k % KD, d * P : (d + 1) * P],
                            h_sb[k][:],
                            start=(k == 0),
                            stop=(k == KI - 1),
                        )
                    yt = ypool.tile([P, cw], F32, tag="y")
                    nc.vector.scalar_tensor_tensor(
                        out=yt[:],
                        in0=ps[:],
                        scalar=b2_dve[:, d : d + 1],
                        in1=g_dve[:, c0 : c0 + cw],
                        op0=mybir.AluOpType.add,
                        op1=mybir.AluOpType.mult,
                    )
                    nc.sync.dma_start(out=yTr[d][:, c0 : c0 + cw], in_=yt[:])
                c0 += cw
    nc.compile()
    return nc


def kernel(**inputs) -> np.ndarray:
    global LAST_EXEC_NS, LAST_RESULTS
    x = np.asarray(inputs["x"], dtype=np.float32)
    Wr = np.asarray(inputs["Wr"], dtype=np.float32)
    br = np.asarray(inputs["br"], dtype=np.float32)
    W1 = np.asarray(inputs["W1"], dtype=np.float32)
    b1 = np.asarray(inputs["b1"], dtype=np.float32)
    W2 = np.asarray(inputs["W2"], dtype=np.float32)
    b2 = np.asarray(inputs["b2"], dtype=np.float32)
    K = int(np.asarray(inputs["top_k"]))

    B, S, D = x.shape
    E = Wr.shape[0]
    I = W1.shape[2]
    T = B * S
    xf = x.reshape(T, D)

    # Router (tiny) on host: logits -> top-k (desc, ties -> lower index,
    # matching jax.lax.top_k) -> softmax over the selected k.
    logits = xf @ Wr.T + br
    order = np.argsort(-logits, axis=-1, kind="stable")[:, :K]
    topv = np.take_along_axis(logits, order, axis=-1)
    exv = np.exp(topv - topv.max(axis=-1, keepdims=True))
    gates = (exv / exv.sum(axis=-1, keepdims=True)).astype(np.float32)

    toks, gvals = [], []
    for e in range(E):
        sel = order == e
        tok = np.nonzero(sel.any(axis=-1))[0]
        kidx = np.argmax(sel[tok], axis=-1)
        toks.append(tok)
        gvals.append(gates[tok, kidx].astype(np.float32))

    maxc = max(max(len(t) for t in toks), P)
    C = ((maxc + P - 1) // P) * P

    key = (C, D, I)
    if key not in _cache:
        _cache[key] = _build(C, D, I)
    nc = _cache[key]

    bf = ml_dtypes.bfloat16
    in_maps = []
    for e in range(E):
        n = len(toks[e])
        xTe = np.zeros((D, C), dtype=bf)
        if n:
            xTe[:, :n] = xf[toks[e]].T.astype(bf)
        ge = np.zeros((P, C), dtype=np.float32)
        if n:
            ge[:, :n] = gvals[e][None, :]
        in_maps.append(
            {
                "xT": xTe,
                "w1": np.ascontiguousarray(W1[e].astype(bf)),
                "b1": np.ascontiguousarray(b1[e].reshape(I // P, P).T),
                "w2": np.ascontiguousarray(W2[e].astype(bf)),
                "b2": np.ascontiguousarray(b2[e].reshape(D // P, P).T),
                "g": ge,
            }
        )

    trace = bool(int(os.environ.get("BASS_KERNEL_TRACE", "0")))
    if trace:
        try:
            from antenv.axon_hooks import get_axon_ntff_profile_hook  # noqa: F401
        except ImportError:
            trace = False
    res = run_bass_kernel_spmd(
        nc, in_maps, core_ids=list(range(N_CORES)), trace=trace
    )
    LAST_EXEC_NS = res.exec_time_ns
    LAST_RESULTS = res

    out = np.zeros((T, D), dtype=np.float32)
    for e in range(E):
        n = len(toks[e])
        if n:
            out[toks[e]] += res.results[e]["yT"][:, :n].T
    return out.reshape(B, S, D)


# revision 13
# speedup vs baseline: 1.0152x; 1.0092x over previous
"""MoE layer (B=4,S=2048,D=1024,I=4096,E=8,top_k=2) on 8 TRN2 NeuronCores.

Strategy: expert-parallel sparse dispatch.
 - Host: router matmul (tiny), top-k + softmax gates, gather tokens per expert.
 - Device (core e == expert e): yT = (gelu(x @ W1) @ W2 + b2) * gate, with
   x/h in bf16, W1 in fp8-e3m4 (x64 scale, descaled for free via the gelu's
   scale operand; adds ~1.4e-2 rel err vs the 2e-2 budget and halves the
   startup-critical weight stream), W2 in bf16, fp32 PSUM accumulation.
   Token dim padded to a multiple of 128, processed in 512-wide chunks.
 - Host: scatter-add the K=2 gated expert outputs back to [B,S,D].

DMA design: descriptor ISSUE on a queue costs ~0.6us regardless of size, and
per-k 2D descriptors were issue-rate bound (the first w1 column regions
didn't finish issuing until ~22us, starving early m1 groups). The host
controls DRAM layout, so x/w1/w2 are packed k-major per partition: each x
chunk, each w1 column region, and each w2 k-group is ONE contiguous 2D
descriptor that transfers at full rate. Head = 3 descriptors.
"""

import os

import ml_dtypes
import numpy as np

import concourse.bass as bass
import concourse.bacc as bacc
import concourse.mybir as mybir
import concourse.tile as tile
from concourse.bass_utils import run_bass_kernel_spmd

BF16 = mybir.dt.bfloat16
F8E3 = mybir.dt.float8e3
F32 = mybir.dt.float32
W1SCALE = 64.0  # w1 stored as e3m4*64 (4-bit mantissa); descaled in the gelu
P = 128
N_CORES = 8
NDUMMY = 13  # HAM warmup matmuls (~427ns each cold, spans until head DMAs land)

# Filled with the profiled exec time (ns) of the last run when
# BASS_KERNEL_TRACE=1 is set in the environment (used by test.py).
LAST_EXEC_NS = None
LAST_RESULTS = None

_cache: dict = {}


def _chunks_for(C: int) -> list[int]:
    """[512, ..., remainder]. Measured faster than equal-width chunks:
    N=512 matmuls hit the 216ns streaming bound and the N=128 tail runs at
    ~56ns/MM (FWL hides LDWEIGHTS), while e.g. N=448 matmuls miss the
    N/2.4+2.5ns model."""
    chunks = [512] * (C // 512)
    if C % 512:
        chunks.append(C % 512)
    return chunks


# w1 column regions: first small so the first m1 groups' weights land early.
# Region r covers m1 output tiles i with 128*i in [a, b).
W1_REGIONS = [(0, 384), (384, 1024), (1024, 2048), (2048, 3072), (3072, 4096)]


def _build(C: int, D: int, I: int):
    """Per-core FFN program: one expert, C token slots (multiple of 128)."""
    KD = D // P  # k-tiles for contraction over D
    KI = I // P  # k-tiles for contraction over I
    ND = D // P  # output row tiles
    NQ = KI // KD  # w2 k-groups

    nc = bacc.Bacc()
    # Host-packed layouts (k-major per partition; see module docstring):
    #  xP[p, chunk-major: (k, c)], w1P[p, region-major: (k, i-a)],
    #  w2P[p, group-major: (k, d)]
    xP = nc.declare_dram_parameter("xP", [P, KD * C], BF16, isOutput=False)
    w1P = nc.declare_dram_parameter("w1P", [P, KD * I], F8E3, isOutput=False)
    b1 = nc.declare_dram_parameter("b1", [P, I // P], F32, isOutput=False)
    w2P = nc.declare_dram_parameter("w2P", [P, KI * D], BF16, isOutput=False)
    b2 = nc.declare_dram_parameter("b2", [P, D // P], F32, isOutput=False)
    g = nc.declare_dram_parameter("g", [P, C], F32, isOutput=False)
    yT = nc.declare_dram_parameter("yT", [D, C], F32, isOutput=True)

    yTr = yT[:].rearrange("(k p) c -> k p c", p=P)

    def w1_slice(w1_sb, i, k):
        """Stationary [128,128] for m1 output tile i, contraction tile k."""
        col = i * P
        for r, (a, b) in enumerate(W1_REGIONS):
            if a <= col < b:
                return w1_sb[r][:, k, col - a : col - a + P]
        raise AssertionError(col)

    with tile.TileContext(nc) as tc:
        with (
            tc.tile_pool(name="wpool", bufs=1) as wpool,
            tc.tile_pool(name="cpool", bufs=1) as cpool,
            tc.tile_pool(name="xpool", bufs=2) as xpool,
            tc.tile_pool(name="hpool", bufs=1) as hpool,
            tc.tile_pool(name="ypool", bufs=4) as ypool,
            tc.tile_pool(name="pspool", bufs=8, space="PSUM") as pspool,
        ):
            chunks = _chunks_for(C)
            # Head: x chunk-0 (1MB) + first w1 region (384KB fp8) + b1, one
            # descriptor each; then the remaining w1 regions stream behind.
            cw0 = chunks[0]
            x0_t = xpool.tile([P, KD, cw0], BF16, tag="x")
            nc.sync.dma_start(out=x0_t[:], in_=xP[:, : KD * cw0])
            w1_sb = []
            a0, b0 = W1_REGIONS[0]
            w1a_t = wpool.tile([P, KD, b0 - a0], F8E3, tag="w1_0")
            nc.sync.dma_start(out=w1a_t[:], in_=w1P[:, : KD * (b0 - a0)])
            w1_sb.append(w1a_t)
            b1_sb = cpool.tile([P, I // P], F32, tag="b1")
            nc.sync.dma_start(out=b1_sb[:], in_=b1[:])
            w1pos = KD * (b0 - a0)
            for r, (a, b) in enumerate(W1_REGIONS[1:], start=1):
                t = wpool.tile([P, KD, b - a], F8E3, tag=f"w1_{r}")
                nc.sync.dma_start(
                    out=t[:], in_=w1P[:, w1pos : w1pos + KD * (b - a)]
                )
                w1pos += KD * (b - a)
                w1_sb.append(t)
            # The Activation encoding fits a single sync wait. Every gelu's
            # PSUM RAW wait (PE sem) dominates its h-slot WAR tick, so the
            # only extra wait a gelu could need is the b1 DMA — absorb it
            # once with a 1-element warm-up copy so ACT's vector clock has
            # observed that DMA before the first real gelu.
            warm = cpool.tile([1, 1], F32, tag="warm")
            warm2 = cpool.tile([1, 1], F32, tag="warm2")
            nc.scalar.copy(warm[:], b1_sb[:1, :1])

            # HAM warm-up: dummy matmuls on zeroed scratch while the head
            # DMAs stream, so real matmuls start at 2.4 GHz instead of
            # paying the 1.2 GHz cold window. Count tuned so the dummies
            # end right as the head data lands (each ~427ns at 1.2GHz).
            scratch = cpool.tile([P, 512], BF16, tag="scratch")
            nc.gpsimd.memset(scratch[:], 0.0)
            left = NDUMMY
            while left > 0:
                grp = min(8, left)
                pw = pspool.tile([P, 512], F32, tag="ps")
                for k in range(grp):
                    nc.tensor.matmul(
                        pw[:],
                        scratch[:, :P],
                        scratch[:],
                        start=(k == 0),
                        stop=(k == grp - 1),
                    )
                left -= grp

            # W2/b2/g are not needed until the first m2 phase — their DMAs
            # are emitted after chunk-0 m1 below so they don't queue ahead of
            # the chunk-0 x tiles in the DMA FIFOs (measured 55 us PE stall).
            w2_sb = []
            b2_dve = None
            g_dve = None

            def _load_phase2():
                for q in range(NQ):
                    t = wpool.tile([P, KD, D], BF16, tag=f"w2_{q}")
                    nc.sync.dma_start(
                        out=t[:],
                        in_=w2P[:, q * KD * D : (q + 1) * KD * D],
                    )
                    w2_sb.append(t)
                b2_sb = cpool.tile([P, D // P], F32, tag="b2")
                nc.sync.dma_start(out=b2_sb[:], in_=b2[:])
                g_sb = cpool.tile([P, C], F32, tag="g")
                nc.sync.dma_start(out=g_sb[:], in_=g[:])
                b2_stage = cpool.tile([P, D // P], F32, tag="b2v")
                nc.vector.tensor_copy(b2_stage[:], b2_sb[:])
                g_stage = cpool.tile([P, C], F32, tag="gv")
                nc.vector.tensor_copy(g_stage[:], g_sb[:])
                return b2_stage, g_stage

            c0 = 0
            xpos = 0
            prev_h_last = None
            for ci, cw in enumerate(chunks):
                if ci == 0:
                    x_t = x0_t
                else:
                    x_t = xpool.tile([P, KD, cw], BF16, tag="x")
                    nc.sync.dma_start(
                        out=x_t[:], in_=xP[:, xpos : xpos + KD * cw]
                    )
                # hT = gelu((x @ W1) / W1SCALE + b1), tiled [128 of I, cw]
                if prev_h_last is not None:
                    # Advance ACT's observed self-tick past ALL of the
                    # previous chunk's gelus so the h-tile WAW deps below
                    # don't each need their own (second) sync wait.
                    nc.scalar.copy(warm[:], prev_h_last[:1, :1])
                    nc.scalar.copy(warm2[:], warm[:])
                h_sb = []
                for i in range(KI):
                    ps = pspool.tile([P, cw], F32, tag="ps")
                    for k in range(KD):
                        nc.tensor.matmul(
                            ps[:],
                            w1_slice(w1_sb, i, k),
                            x_t[:, k, :],
                            start=(k == 0),
                            stop=(k == KD - 1),
                        )
                    ht = hpool.tile([P, cw], BF16, tag=f"h_{i}")
                    nc.scalar.activation(
                        ht[:],
                        ps[:],
                        mybir.ActivationFunctionType.Gelu,
                        bias=b1_sb[:, i : i + 1],
                        scale=1.0 / W1SCALE,
                    )
                    h_sb.append(ht)
                prev_h_last = h_sb[-1]
                if g_dve is None:
                    b2_dve, g_dve = _load_phase2()
                # yT = (hT' @ W2 + b2) * g, tiled [128 of D, cw]
                for d in range(ND):
                    ps = pspool.tile([P, cw], F32, tag="ps")
                    for k in range(KI):
                        nc.tensor.matmul(
                            ps[:],
                            w2_sb[k // KD][:, k % KD, d * P : (d + 1) * P],
                            h_sb[k][:],
                            start=(k == 0),
                            stop=(k == KI - 1),
                        )
                    yt = ypool.tile([P, cw], F32, tag="y")
                    nc.vector.scalar_tensor_tensor(
                        out=yt[:],
                        in0=ps[:],
                        scalar=b2_dve[:, d : d + 1],
                        in1=g_dve[:, c0 : c0 + cw],
                        op0=mybir.AluOpType.add,
                        op1=mybir.AluOpType.mult,
                    )
                    nc.sync.dma_start(out=yTr[d][:, c0 : c0 + cw], in_=yt[:])
                c0 += cw
                xpos += KD * cw
    nc.compile()
    return nc


def _pack_x(xTe: np.ndarray, chunks: list[int], KD: int) -> np.ndarray:
    """[D, C] -> [P, KD*C] packed chunk-major, k-major within chunk."""
    D, C = xTe.shape
    kv = xTe.reshape(KD, P, C)
    out = np.empty((P, KD * C), dtype=xTe.dtype)
    pos = 0
    c0 = 0
    for cw in chunks:
        blk = kv[:, :, c0 : c0 + cw].transpose(1, 0, 2).reshape(P, KD * cw)
        out[:, pos : pos + KD * cw] = blk
        pos += KD * cw
        c0 += cw
    return out


def _pack_w1(w1e: np.ndarray, KD: int) -> np.ndarray:
    """[D, I] -> [P, KD*I] packed region-major, k-major within region."""
    D, I = w1e.shape
    kv = w1e.reshape(KD, P, I)
    out = np.empty((P, KD * I), dtype=w1e.dtype)
    pos = 0
    for a, b in W1_REGIONS:
        w = b - a
        out[:, pos : pos + KD * w] = (
            kv[:, :, a:b].transpose(1, 0, 2).reshape(P, KD * w)
        )
        pos += KD * w
    return out


def _pack_w2(w2e: np.ndarray, KI: int, KD: int) -> np.ndarray:
    """[I, D] -> [P, KI*D] packed k-group-major, k-major within group."""
    I, D = w2e.shape
    kv = w2e.reshape(KI, P, D)
    out = np.empty((P, KI * D), dtype=w2e.dtype)
    pos = 0
    for q in range(KI // KD):
        out[:, pos : pos + KD * D] = (
            kv[q * KD : (q + 1) * KD].transpose(1, 0, 2).reshape(P, KD * D)
        )
        pos += KD * D
    return out


def kernel(**inputs) -> np.ndarray:
    global LAST_EXEC_NS, LAST_RESULTS
    x = np.asarray(inputs["x"], dtype=np.float32)
    Wr = np.asarray(inputs["Wr"], dtype=np.float32)
    br = np.asarray(inputs["br"], dtype=np.float32)
    W1 = np.asarray(inputs["W1"], dtype=np.float32)
    b1 = np.asarray(inputs["b1"], dtype=np.float32)
    W2 = np.asarray(inputs["W2"], dtype=np.float32)
    b2 = np.asarray(inputs["b2"], dtype=np.float32)
    K = int(np.asarray(inputs["top_k"]))

    B, S, D = x.shape
    E = Wr.shape[0]
    I = W1.shape[2]
    T = B * S
    KD = D // P
    KI = I // P
    xf = x.reshape(T, D)

    # Router (tiny) on host: logits -> top-k (desc, ties -> lower index,
    # matching jax.lax.top_k) -> softmax over the selected k.
    logits = xf @ Wr.T + br
    order = np.argsort(-logits, axis=-1, kind="stable")[:, :K]
    topv = np.take_along_axis(logits, order, axis=-1)
    exv = np.exp(topv - topv.max(axis=-1, keepdims=True))
    gates = (exv / exv.sum(axis=-1, keepdims=True)).astype(np.float32)

    toks, gvals = [], []
    for e in range(E):
        sel = order == e
        tok = np.nonzero(sel.any(axis=-1))[0]
        kidx = np.argmax(sel[tok], axis=-1)
        toks.append(tok)
        gvals.append(gates[tok, kidx].astype(np.float32))

    maxc = max(max(len(t) for t in toks), P)
    C = ((maxc + P - 1) // P) * P
    chunks = _chunks_for(C)

    key = (C, D, I)
    if key not in _cache:
        _cache[key] = _build(C, D, I)
    nc = _cache[key]

    bf = ml_dtypes.bfloat16
    f8 = ml_dtypes.float8_e3m4
    in_maps = []
    for e in range(E):
        n = len(toks[e])
        xTe = np.zeros((D, C), dtype=bf)
        if n:
            xTe[:, :n] = xf[toks[e]].T.astype(bf)
        ge = np.zeros((P, C), dtype=np.float32)
        if n:
            ge[:, :n] = gvals[e][None, :]
        in_maps.append(
            {
                "xP": _pack_x(xTe, chunks, KD),
                "w1P": _pack_w1((W1[e] * W1SCALE).astype(f8), KD),
                "b1": np.ascontiguousarray(b1[e].reshape(I // P, P).T),
                "w2P": _pack_w2(W2[e].astype(bf), KI, KD),
                "b2": np.ascontiguousarray(b2[e].reshape(D // P, P).T),
                "g": ge,
            }
        )

    trace = bool(int(os.environ.get("BASS_KERNEL_TRACE", "0")))
    if trace:
        try:
            from antenv.axon_hooks import get_axon_ntff_profile_hook  # noqa: F401
        except ImportError:
            trace = False
    res = run_bass_kernel_spmd(
        nc, in_maps, core_ids=list(range(N_CORES)), trace=trace
    )
    LAST_EXEC_NS = res.exec_time_ns
    LAST_RESULTS = res

    out = np.zeros((T, D), dtype=np.float32)
    for e in range(E):
        n = len(toks[e])
        if n:
            out[toks[e]] += res.results[e]["yT"][:, :n].T
    return out.reshape(B, S, D)
